# revision 11
# baseline (speedup 1.0000x reference)
"""Trainium2 Bass kernel for the batched attention-context module.

Math (per batch b):
    energy[l]  = dot(current_hidden[b], encoder_outputs[b, l])      # [L]
    align      = softmax(energy)                                    # [L]
    context[d] = sum_l align[l] * encoder_outputs[b, l, d] / L      # [D]

Sharding: data-parallel over batch, 8 batches per NeuronCore, 8 cores.
Single pass over encoder_outputs (512MB total): each chunk of a batch's
E is DMA'd into SBUF once and used for both the energy dot products
(VectorE fused multiply+reduce) and the context weighted sum (TensorE
matmuls, float32r streaming, with the softmax weights as stationary).

Softmax uses a constant shift instead of the data max (shift-invariant;
energies are dots of 512 N(0,1) pairs, std ~22.6, so exp(e-64) spans
~[e^-160, e^45] — comfortably inside fp32). Because the shift is a
constant, the exp weights and the context accumulation need no global
statistic: everything pipelines at chunk granularity and only the final
1/(denom*L) scale waits for the whole batch.
"""

from contextlib import ExitStack

import numpy as np

B, L, D = 64, 4096, 512
N_CORES = 8
B_LOC = B // N_CORES          # 8 batches per core
P = 128                       # partitions
SHIFT = 64.0                  # constant softmax shift
CHUNK_T = 8                   # l-tiles (of 128) per DMA/compute chunk

_BUILD_CACHE = {}


def build_nc(b_loc=B_LOC, seq=L, dim=D, e_bufs=8, verbose=False):
    import time as _time

    import concourse.tile as tile
    from concourse import bacc, mybir

    _t0 = _time.monotonic()

    def _mark(msg):
        if verbose:
            print(f"[build {_time.monotonic() - _t0:7.1f}s] {msg}", flush=True)

    FP32 = mybir.dt.float32
    FP32R = mybir.dt.float32r
    Alu = mybir.AluOpType
    Act = mybir.ActivationFunctionType
    T = seq // P                      # l-tiles per batch
    CT = min(CHUNK_T, T)              # tiles per chunk
    NCH = (T + CT - 1) // CT          # chunks per batch
    assert T % CT == 0

    _mark("start")
    nc = bacc.Bacc("TRN2", target_bir_lowering=False, debug=False)
    enc = nc.dram_tensor("enc", [b_loc, seq, dim], FP32, kind="ExternalInput").ap()
    hrep = nc.dram_tensor("hrep", [b_loc, P, dim], FP32, kind="ExternalInput").ap()
    ones = nc.dram_tensor("ones", [P, 2], FP32, kind="ExternalInput").ap()
    out = nc.dram_tensor("out", [b_loc, dim], FP32, kind="ExternalOutput").ap()

    with tile.TileContext(nc) as tc, ExitStack() as ctx:
        e_pool = ctx.enter_context(tc.tile_pool(name="e", bufs=e_bufs))
        h_pool = ctx.enter_context(tc.tile_pool(name="h", bufs=1))
        scr_pool = ctx.enter_context(tc.tile_pool(name="scr", bufs=2))
        stat_pool = ctx.enter_context(tc.tile_pool(name="stat", bufs=3))
        out_pool = ctx.enter_context(tc.tile_pool(name="o", bufs=2))
        psum_pool = ctx.enter_context(tc.tile_pool(name="ps", bufs=2, space="PSUM"))
        psum_sm = ctx.enter_context(tc.tile_pool(name="pss", bufs=3, space="PSUM"))

        cn = h_pool.tile([P, 2], FP32)
        nc.sync.dma_start(cn[:], ones[:])
        ones_col = cn[:, 0:1]
        negshift = cn[:, 1:2]

        h_sb = h_pool.tile([P, b_loc, dim], FP32)
        nc.sync.dma_start(h_sb[:], hrep.rearrange("b p d -> p b d"))

        # DRAM view: chunk c of batch b = rows [c*CT*P, (c+1)*CT*P)
        enc_v = enc.rearrange("b (c t p) d -> b c p t d", p=P, t=CT)

        for b in range(b_loc):
            e_buf = stat_pool.tile([P, T], FP32, tag="ebuf")
            w_buf = stat_pool.tile([P, T], FP32, tag="wbuf")
            s1c = stat_pool.tile([P, NCH], FP32, tag="s1c")
            ps = psum_pool.tile([1, dim], FP32, tag="ps")

            for c in range(NCH):
                e_sb = e_pool.tile([P, CT, dim], FP32, tag="esb")
                nc.sync.dma_start(e_sb[:], enc_v[b, c])

                # energy: fused multiply + reduce per l-tile
                scr = scr_pool.tile([P, dim], FP32, tag="scr")
                for t in range(CT):
                    nc.vector.scalar_tensor_tensor(
                        out=scr[:],
                        in0=e_sb[:, t, :],
                        scalar=1.0,
                        in1=h_sb[:, b, :],
                        op0=Alu.mult,
                        op1=Alu.mult,
                        accum_out=e_buf[:, c * CT + t : c * CT + t + 1],
                    )

                # w = exp(e - SHIFT) for this chunk, with fused row-sum
                nc.scalar.activation(
                    w_buf[:, c * CT : (c + 1) * CT],
                    e_buf[:, c * CT : (c + 1) * CT],
                    Act.Exp,
                    bias=negshift,
                    scale=1.0,
                    accum_out=s1c[:, c : c + 1],
                )

                # context partial: ps += w[:, t].T @ E_t  (float32r stream)
                for t in range(CT):
                    g = c * CT + t
                    nc.tensor.matmul(
                        ps[:],
                        w_buf[:, g : g + 1],
                        e_sb[:, t, :],
                        start=(g == 0),
                        stop=(g == T - 1),
                    )

            # denominator and final scale
            s1 = stat_pool.tile([P, 1], FP32, tag="s1")
            nc.vector.tensor_reduce(
                s1[:], s1c[:], axis=mybir.AxisListType.X, op=Alu.add
            )
            den_ps = psum_sm.tile([1, 1], FP32, tag="denps")
            nc.tensor.matmul(den_ps[:], s1[:], ones_col, start=True, stop=True)
            rcp = stat_pool.tile([1, 1], FP32, tag="rcp")
            nc.vector.reciprocal(rcp[:], den_ps[:])
            scale_s = stat_pool.tile([1, 1], FP32, tag="scales")
            nc.vector.tensor_scalar_mul(scale_s[:], rcp[:], 1.0 / seq)

            out_row = out_pool.tile([1, dim], FP32, tag="orow")
            nc.scalar.activation(out_row[:], ps[:], Act.Copy, scale=scale_s[:])
            nc.scalar.dma_start(out[b : b + 1, :], out_row[:])

    _mark("tile traced+scheduled")
    nc.compile()
    _mark("bacc compiled")
    return nc


def make_in_maps(current_hidden, encoder_outputs, b_loc=B_LOC, n_cores=N_CORES):
    current_hidden = np.asarray(current_hidden, dtype=np.float32)
    encoder_outputs = np.asarray(encoder_outputs, dtype=np.float32)
    dim = current_hidden.shape[-1]
    ones = np.ones((P, 2), np.float32)
    ones[:, 1] = -SHIFT
    in_maps = []
    for c in range(n_cores):
        lo, hi = c * b_loc, (c + 1) * b_loc
        hc = current_hidden[lo:hi]
        in_maps.append(
            {
                "enc": np.ascontiguousarray(encoder_outputs[lo:hi]),
                "hrep": np.ascontiguousarray(
                    np.broadcast_to(hc[:, None, :], (b_loc, P, dim))
                ),
                "ones": ones,
            }
        )
    return in_maps


def _get_nc():
    if "nc" not in _BUILD_CACHE:
        _BUILD_CACHE["nc"] = build_nc()
    return _BUILD_CACHE["nc"]


def kernel(current_hidden, encoder_outputs):
    from concourse.bass_utils import run_bass_kernel_spmd

    nc = _get_nc()
    in_maps = make_in_maps(current_hidden, encoder_outputs)
    res = run_bass_kernel_spmd(nc, in_maps, core_ids=list(range(N_CORES)))
    out = np.concatenate(
        [res.results[c]["out"] for c in range(N_CORES)], axis=0
    )
    return out.astype(np.float32)


# revision 12
# speedup vs baseline: 1.2246x; 1.2246x over previous
"""Trainium2 Bass kernel for the batched attention-context module.

Math (per batch b):
    energy[l]  = dot(current_hidden[b], encoder_outputs[b, l])      # [L]
    align      = softmax(energy)                                    # [L]
    context[d] = sum_l align[l] * encoder_outputs[b, l, d] / L      # [D]

Sharding: data-parallel over batch, 8 batches per NeuronCore, 8 cores.
Single pass over encoder_outputs (512MB total): each chunk of a batch's
E is DMA'd into SBUF once and used for both the energy dot products
(VectorE fused multiply+reduce) and the context weighted sum (TensorE
matmuls, float32r streaming, with the softmax weights as stationary).

Softmax uses a constant shift instead of the data max (shift-invariant;
energies are dots of 512 N(0,1) pairs, std ~22.6, so exp(e-64) spans
~[e^-160, e^45] — comfortably inside fp32). Because the shift is a
constant, the exp weights and the context accumulation need no global
statistic: everything pipelines at chunk granularity and only the final
1/(denom*L) scale waits for the whole batch.
"""

from contextlib import ExitStack

import numpy as np

B, L, D = 64, 4096, 512
N_CORES = 8
B_LOC = B // N_CORES          # 8 batches per core
P = 128                       # partitions
SHIFT = 64.0                  # constant softmax shift
CHUNK_T = 8                   # l-tiles (of 128) per DMA/compute chunk

_BUILD_CACHE = {}


def build_nc(b_loc=B_LOC, seq=L, dim=D, e_bufs=8, verbose=False):
    import time as _time

    import concourse.tile as tile
    from concourse import bacc, mybir

    _t0 = _time.monotonic()

    def _mark(msg):
        if verbose:
            print(f"[build {_time.monotonic() - _t0:7.1f}s] {msg}", flush=True)

    FP32 = mybir.dt.float32
    FP32R = mybir.dt.float32r
    Alu = mybir.AluOpType
    Act = mybir.ActivationFunctionType
    T = seq // P                      # l-tiles per batch
    CT = min(CHUNK_T, T)              # tiles per chunk
    NCH = (T + CT - 1) // CT          # chunks per batch
    assert T % CT == 0

    _mark("start")
    nc = bacc.Bacc("TRN2", target_bir_lowering=False, debug=False)
    enc = nc.dram_tensor("enc", [b_loc, seq, dim], FP32, kind="ExternalInput").ap()
    hrep = nc.dram_tensor("hrep", [b_loc, P, dim], FP32, kind="ExternalInput").ap()
    ones = nc.dram_tensor("ones", [P, 2], FP32, kind="ExternalInput").ap()
    out = nc.dram_tensor("out", [b_loc, dim], FP32, kind="ExternalOutput").ap()

    with tile.TileContext(nc) as tc, ExitStack() as ctx:
        e_pool = ctx.enter_context(tc.tile_pool(name="e", bufs=e_bufs))
        h_pool = ctx.enter_context(tc.tile_pool(name="h", bufs=1))
        scr_pool = ctx.enter_context(tc.tile_pool(name="scr", bufs=2))
        stat_pool = ctx.enter_context(tc.tile_pool(name="stat", bufs=3))
        out_pool = ctx.enter_context(tc.tile_pool(name="o", bufs=2))
        psum_pool = ctx.enter_context(tc.tile_pool(name="ps", bufs=2, space="PSUM"))
        psum_sm = ctx.enter_context(tc.tile_pool(name="pss", bufs=3, space="PSUM"))

        cn = h_pool.tile([P, 2], FP32)
        nc.sync.dma_start(cn[:], ones[:])
        ones_col = cn[:, 0:1]
        negshift = cn[:, 1:2]

        h_sb = h_pool.tile([P, b_loc, dim], FP32)
        nc.sync.dma_start(h_sb[:], hrep.rearrange("b p d -> p b d"))

        # DRAM view: chunk c of batch b = rows [c*CT*P, (c+1)*CT*P)
        enc_v = enc.rearrange("b (c t p) d -> b c p t d", p=P, t=CT)

        for b in range(b_loc):
            e_buf = stat_pool.tile([P, T], FP32, tag="ebuf")
            w_buf = stat_pool.tile([P, T], FP32R, tag="wbuf")
            s1c = stat_pool.tile([P, NCH], FP32, tag="s1c")
            ps = psum_pool.tile([1, dim], FP32, tag="ps")

            for c in range(NCH):
                e_sb = e_pool.tile([P, CT, dim], FP32R, tag="esb")
                nc.sync.dma_start(e_sb[:], enc_v[b, c].bitcast(FP32R))

                # energy: fused multiply + reduce per l-tile
                scr = scr_pool.tile([P, dim], FP32, tag="scr")
                for t in range(CT):
                    nc.vector.scalar_tensor_tensor(
                        out=scr[:],
                        in0=e_sb[:, t, :].bitcast(FP32),
                        scalar=1.0,
                        in1=h_sb[:, b, :],
                        op0=Alu.mult,
                        op1=Alu.mult,
                        accum_out=e_buf[:, c * CT + t : c * CT + t + 1],
                    )

                # w = exp(e - SHIFT) for this chunk, with fused row-sum
                nc.scalar.activation(
                    w_buf[:, c * CT : (c + 1) * CT],
                    e_buf[:, c * CT : (c + 1) * CT],
                    Act.Exp,
                    bias=negshift,
                    scale=1.0,
                    accum_out=s1c[:, c : c + 1],
                )

                # context partial: ps += w[:, t].T @ E_t  (float32r stream)
                for t in range(CT):
                    g = c * CT + t
                    nc.tensor.matmul(
                        ps[:],
                        w_buf[:, g : g + 1],
                        e_sb[:, t, :],
                        start=(g == 0),
                        stop=(g == T - 1),
                    )

            # denominator and final scale
            s1 = stat_pool.tile([P, 1], FP32, tag="s1")
            nc.vector.tensor_reduce(
                s1[:], s1c[:], axis=mybir.AxisListType.X, op=Alu.add
            )
            den_ps = psum_sm.tile([1, 1], FP32, tag="denps")
            nc.tensor.matmul(den_ps[:], s1[:], ones_col, start=True, stop=True)
            rcp = stat_pool.tile([1, 1], FP32, tag="rcp")
            nc.vector.reciprocal(rcp[:], den_ps[:])
            scale_s = stat_pool.tile([1, 1], FP32, tag="scales")
            nc.vector.tensor_scalar_mul(scale_s[:], rcp[:], 1.0 / seq)

            out_row = out_pool.tile([1, dim], FP32, tag="orow")
            nc.scalar.activation(out_row[:], ps[:], Act.Copy, scale=scale_s[:])
            nc.scalar.dma_start(out[b : b + 1, :], out_row[:])

    _mark("tile traced+scheduled")
    nc.compile()
    _mark("bacc compiled")
    return nc


def make_in_maps(current_hidden, encoder_outputs, b_loc=B_LOC, n_cores=N_CORES):
    current_hidden = np.asarray(current_hidden, dtype=np.float32)
    encoder_outputs = np.asarray(encoder_outputs, dtype=np.float32)
    dim = current_hidden.shape[-1]
    ones = np.ones((P, 2), np.float32)
    ones[:, 1] = -SHIFT
    in_maps = []
    for c in range(n_cores):
        lo, hi = c * b_loc, (c + 1) * b_loc
        hc = current_hidden[lo:hi]
        in_maps.append(
            {
                "enc": np.ascontiguousarray(encoder_outputs[lo:hi]),
                "hrep": np.ascontiguousarray(
                    np.broadcast_to(hc[:, None, :], (b_loc, P, dim))
                ),
                "ones": ones,
            }
        )
    return in_maps


def _get_nc():
    if "nc" not in _BUILD_CACHE:
        _BUILD_CACHE["nc"] = build_nc()
    return _BUILD_CACHE["nc"]


def kernel(current_hidden, encoder_outputs):
    from concourse.bass_utils import run_bass_kernel_spmd

    nc = _get_nc()
    in_maps = make_in_maps(current_hidden, encoder_outputs)
    res = run_bass_kernel_spmd(nc, in_maps, core_ids=list(range(N_CORES)))
    out = np.concatenate(
        [res.results[c]["out"] for c in range(N_CORES)], axis=0
    )
    return out.astype(np.float32)
